# revision 8
# baseline (speedup 1.0000x reference)
"""Grouped self-attention (GQA) Trainium2 kernel.

Problem: B=2, T=2048, D=2048, 16 Q heads / 4 KV heads, head_dim=128,
full RoPE (base 1e6), causal softmax, output projection.

Sharding: 8 cores = 2 batches x 4 KV groups. Core c handles batch c//4,
kv-group c%4 (4 Q heads + 1 KV head). q/k/v projections column-sharded,
o_proj row-sharded; per-core partial outputs are summed on host.

Per-core pipeline (all matmuls fp32r except P@V in bf16):
  phase 1: qT/kT/vT = W.T @ x.T (x pre-transposed on host), RoPE fused
           on PSUM->SBUF eviction, v transposed to [tk, d] blocks.
  phase 2: per 128-row q block i, per head h:
           S = qT_h.T @ kT (causal blocks only), mask diag, exp on ACT
           (scale=1/sqrt(d), accum_out=row sums), P^T via PE transpose,
           O = P^T.T @ V accumulated over tk blocks, normalize by
           1/rowsum on PSUM eviction, O^T via PE transpose, then
           Y_partial = O^T.T @ Wo_g streamed straight from PSUM to DRAM.
"""

import os
import sys

import numpy as np

for _p in ("/opt/trn_rl_repo",):
    if _p not in sys.path and os.path.isdir(_p):
        sys.path.insert(0, _p)

import concourse.bass as bass  # noqa: E402
import concourse.mybir as mybir  # noqa: E402
import concourse.tile as tile  # noqa: E402
from concourse import bacc  # noqa: E402
from concourse.bass_utils import run_bass_kernel_spmd  # noqa: E402
from concourse.masks import make_identity  # noqa: E402

B, T, D = 2, 2048, 2048
NH, NKV, HD = 16, 4, 128
G = NKV              # kv groups == cores per batch
AQ = (NH // NKV) * HD  # attention cols per core (4 heads x 128)
NQB = T // 128       # 16 q blocks
KC = D // 128        # 16 contraction chunks for projections
ROPE_BASE = 1000000.0
INV_SQRT_D = 1.0 / float(np.sqrt(HD))

F32 = mybir.dt.float32
F32R = mybir.dt.float32r
BF16 = mybir.dt.bfloat16

_CACHE = {}


def _build_nc():
    nc = bacc.Bacc(None, target_bir_lowering=False, debug=False)

    xT_d = nc.dram_tensor("xT", [D, T], F32R, kind="ExternalInput")
    wq_d = nc.dram_tensor("wq", [D, AQ], F32R, kind="ExternalInput")
    wk_d = nc.dram_tensor("wk", [D, HD], F32R, kind="ExternalInput")
    wv_d = nc.dram_tensor("wv", [D, HD], F32R, kind="ExternalInput")
    wo_d = nc.dram_tensor("wo", [AQ, D], F32R, kind="ExternalInput")
    cos_d = nc.dram_tensor("cosT", [HD, T], F32, kind="ExternalInput")
    sin_d = nc.dram_tensor("sinT", [HD, T], F32, kind="ExternalInput")
    mask_d = nc.dram_tensor("mask", [128, 128], F32, kind="ExternalInput")
    y_d = nc.dram_tensor("y", [T, D], F32, kind="ExternalOutput")

    mult = mybir.AluOpType.mult
    add = mybir.AluOpType.add
    Exp = mybir.ActivationFunctionType.Exp
    Copy = mybir.ActivationFunctionType.Copy

    with tile.TileContext(nc) as tc:
        with (
            tc.tile_pool(name="const", bufs=1) as cpool,
            tc.tile_pool(name="qkv", bufs=1) as qkv_pool,
        ):
            cos_sb = cpool.tile([HD, T], F32, tag="cos")
            sin_sb = cpool.tile([HD, T], F32, tag="sin")
            mask_sb = cpool.tile([128, 128], F32, tag="mask")
            id_bf = cpool.tile([128, 128], BF16, tag="idb")
            id_f32 = cpool.tile([128, 128], F32, tag="idf")
            nc.sync.dma_start(cos_sb[:], cos_d[:])
            nc.sync.dma_start(sin_sb[:], sin_d[:])
            nc.sync.dma_start(mask_sb[:], mask_d[:])
            make_identity(nc, id_bf[:])
            make_identity(nc, id_f32[:])

            qT = qkv_pool.tile([128, 4, T], F32R, tag="qT")    # [d, h, t]
            kT = qkv_pool.tile([128, T], F32R, tag="kT")       # [d, t]
            v_sb = qkv_pool.tile([128, T], BF16, tag="v")     # [tk%128, blk*128+d]

            # ---------------- phase 1: projections + rope ----------------
            with (
                tc.tile_pool(name="xt", bufs=1) as xt_pool,
                tc.tile_pool(name="wld", bufs=4) as w_pool,
                tc.tile_pool(name="p1ps", bufs=1, space="PSUM") as pps,
                tc.tile_pool(name="p1vt", bufs=2, space="PSUM") as pvt,
                tc.tile_pool(name="p1tmp", bufs=3) as tmp_pool,
            ):
                xt = xt_pool.tile([128, KC, T], F32R, tag="xt")
                for e in range(KC):
                    nc.sync.dma_start(xt[:, e, :], xT_d[e * 128:(e + 1) * 128, :])

                for s in range(6):
                    if s < 4:
                        src = wq_d[:, s * 128:(s + 1) * 128]
                    elif s == 4:
                        src = wk_d[:, :]
                    else:
                        src = wv_d[:, :]
                    pss = [pps.tile([128, 512], F32, tag=f"proj{t}",
                                    name=f"proj_{s}_{t}")
                           for t in range(4)]
                    for e in range(KC):
                        we = w_pool.tile([128, 128], F32R, tag="w")
                        nc.sync.dma_start(we[:], src[e * 128:(e + 1) * 128, :])
                        for tci in range(4):
                            nc.tensor.matmul(
                                pss[tci][:],
                                we[:],
                                xt[:, e, tci * 512:(tci + 1) * 512],
                                start=(e == 0),
                                stop=(e == KC - 1),
                            )
                    for tci in range(4):
                        tsl = slice(tci * 512, (tci + 1) * 512)
                        ps = pss[tci]
                        if s < 5:
                            dst = qT[:, s, tsl] if s < 4 else kT[:, tsl]
                            t1 = tmp_pool.tile([128, 512], F32, tag="ropetmp")
                            nc.vector.tensor_tensor(t1[:], ps[:], cos_sb[:, tsl], mult)
                            nc.vector.tensor_tensor(
                                dst[0:64, :], ps[64:128, :], sin_sb[0:64, tsl], mult)
                            nc.vector.tensor_tensor(
                                dst[64:128, :], ps[0:64, :], sin_sb[64:128, tsl], mult)
                            nc.vector.tensor_tensor(dst[:], dst[:], t1[:], add)
                        else:
                            # vT chunk [d, t512] -> bf16, then transpose to v blocks
                            vt = tmp_pool.tile([128, 512], BF16, tag="vtmp")
                            nc.scalar.copy(vt[:], ps[:])
                            pst = pvt.tile([128, 512], BF16, tag="vtr")
                            for j4 in range(4):
                                nc.tensor.transpose(
                                    pst[:, j4 * 128:(j4 + 1) * 128],
                                    vt[:, j4 * 128:(j4 + 1) * 128],
                                    id_bf[:],
                                )
                            nc.vector.tensor_copy(v_sb[:, tsl], pst[:])

            # ---------------- phase 2: attention + o-proj ----------------
            with (
                tc.tile_pool(name="wop", bufs=1) as wo_pool,
                tc.tile_pool(name="att", bufs=2) as att_pool,
                tc.tile_pool(name="small", bufs=6) as small_pool,
                tc.tile_pool(name="ps_s", bufs=2, space="PSUM") as ps_s_pool,
                tc.tile_pool(name="ps_t", bufs=2, space="PSUM") as ps_t_pool,
                tc.tile_pool(name="ps_o", bufs=2, space="PSUM") as ps_o_pool,
                tc.tile_pool(name="ps_y", bufs=2, space="PSUM") as ps_y_pool,
            ):
                wo_sb = wo_pool.tile([128, 4, D], F32R, tag="wo")
                for h in range(4):
                    nc.sync.dma_start(
                        wo_sb[:, h, :], wo_d[h * 128:(h + 1) * 128, :])

                cp = 0
                for i in range(NQB):
                    L = (i + 1) * 128
                    ncnk = (L + 511) // 512
                    O_sb = small_pool.tile([128, 4, 128], F32, tag="Osb")
                    for h in range(4):
                        P = att_pool.tile([128, T], BF16, tag="P")
                        sums = small_pool.tile([128, 4], F32, tag="sums")
                        for c in range(ncnk):
                            cl = min(512, L - c * 512)
                            ps_s = ps_s_pool.tile([128, 512], F32, tag="S")
                            nc.tensor.matmul(
                                ps_s[:, :cl],
                                qT[:, h, i * 128:(i + 1) * 128],
                                kT[:, c * 512:c * 512 + cl],
                                start=True,
                                stop=True,
                            )
                            if c == ncnk - 1:
                                nc.vector.tensor_tensor(
                                    ps_s[:, cl - 128:cl], ps_s[:, cl - 128:cl],
                                    mask_sb[:], add)
                            nc.scalar.activation(
                                P[:, c * 512:c * 512 + cl],
                                ps_s[:, :cl],
                                Exp,
                                scale=INV_SQRT_D,
                                accum_out=sums[:, c:c + 1],
                            )
                        rs = small_pool.tile([128, 1], F32, tag="rs")
                        if ncnk > 1:
                            tot = small_pool.tile([128, 1], F32, tag="tot")
                            nc.vector.tensor_reduce(
                                tot[:], sums[:, :ncnk],
                                axis=mybir.AxisListType.X, op=add)
                            nc.vector.reciprocal(rs[:], tot[:])
                        else:
                            nc.vector.reciprocal(rs[:], sums[:, 0:1])

                        PT = att_pool.tile([128, T], BF16, tag="PT")
                        for jb in range(0, i + 1, 4):
                            jn = min(4, i + 1 - jb)
                            ps_t = ps_t_pool.tile([128, 512], BF16, tag="PTb")
                            for j4 in range(jn):
                                nc.tensor.transpose(
                                    ps_t[:, j4 * 128:(j4 + 1) * 128],
                                    P[:, (jb + j4) * 128:(jb + j4 + 1) * 128],
                                    id_bf[:],
                                )
                            dst = PT[:, jb * 128:(jb + jn) * 128]
                            if cp % 2 == 0:
                                nc.scalar.copy(dst, ps_t[:, :jn * 128])
                            else:
                                nc.vector.tensor_copy(dst, ps_t[:, :jn * 128])
                            cp += 1

                        ps_o = ps_o_pool.tile([128, 128], F32, tag="O")
                        for j in range(i + 1):
                            nc.tensor.matmul(
                                ps_o[:],
                                PT[:, j * 128:(j + 1) * 128],
                                v_sb[:, j * 128:(j + 1) * 128],
                                start=(j == 0),
                                stop=(j == i),
                            )
                        nc.scalar.activation(
                            O_sb[:, h, :], ps_o[:], Copy, scale=rs[:])

                    ps_ot = ps_t_pool.tile([128, 512], F32, tag="PTp", bufs=1)
                    for h in range(4):
                        nc.tensor.transpose(
                            ps_ot[:, h * 128:(h + 1) * 128], O_sb[:, h, :],
                            id_f32[:])
                    OT_sb = small_pool.tile([128, 512], F32R, tag="OT")
                    nc.vector.tensor_copy(OT_sb[:], ps_ot[:])

                    for nci in range(4):
                        ps_y = ps_y_pool.tile([128, 512], F32, tag="Y", bufs=1)
                        for h in range(4):
                            nc.tensor.matmul(
                                ps_y[:],
                                OT_sb[:, h * 128:(h + 1) * 128],
                                wo_sb[:, h, nci * 512:(nci + 1) * 512],
                                start=(h == 0),
                                stop=(h == 3),
                            )
                        y_sb = att_pool.tile([128, 512], F32, tag="ysb")
                        if cp % 2 == 0:
                            nc.scalar.copy(y_sb[:], ps_y[:])
                        else:
                            nc.vector.tensor_copy(y_sb[:], ps_y[:])
                        cp += 1
                        nc.sync.dma_start(
                            y_d[i * 128:(i + 1) * 128, nci * 512:(nci + 1) * 512],
                            y_sb[:])

    nc.compile()
    return nc


def _rope_tables():
    # match reference float32 arithmetic exactly
    pos = np.arange(T, dtype=np.float32)
    inv_freq = (1.0 / (ROPE_BASE ** (np.arange(0, HD, 2, dtype=np.float32) / HD))).astype(np.float32)
    ang = pos[:, None] * inv_freq[None, :]            # [T, 64]
    cos = np.cos(ang).astype(np.float32)
    sin = np.sin(ang).astype(np.float32)
    cosT = np.ascontiguousarray(np.concatenate([cos, cos], 1).T)   # [128, T]
    sinT = np.ascontiguousarray(np.concatenate([-sin, sin], 1).T)  # rotate_half sign
    return cosT, sinT


def kernel(x, Wq, bq, Wk, bk, Wv, bv, Wo, bo, **_ignored):
    x = np.asarray(x, dtype=np.float32)
    Wq = np.asarray(Wq, dtype=np.float32)
    Wk = np.asarray(Wk, dtype=np.float32)
    Wv = np.asarray(Wv, dtype=np.float32)
    Wo = np.asarray(Wo, dtype=np.float32)
    bo = np.asarray(bo, dtype=np.float32)

    if "nc" not in _CACHE:
        _CACHE["nc"] = _build_nc()
    nc = _CACHE["nc"]

    cosT, sinT = _rope_tables()
    tri = np.tril(np.ones((128, 128), dtype=bool))
    mask = np.where(tri, 0.0, -1e9).astype(np.float32)

    in_maps = []
    for c in range(8):
        b, g = c // G, c % G
        in_maps.append({
            "xT": np.ascontiguousarray(x[b].T),
            "wq": np.ascontiguousarray(Wq[:, g * AQ:(g + 1) * AQ]),
            "wk": np.ascontiguousarray(Wk[:, g * HD:(g + 1) * HD]),
            "wv": np.ascontiguousarray(Wv[:, g * HD:(g + 1) * HD]),
            "wo": np.ascontiguousarray(Wo[g * AQ:(g + 1) * AQ, :]),
            "cosT": cosT,
            "sinT": sinT,
            "mask": mask,
        })

    res = run_bass_kernel_spmd(
        nc, in_maps, list(range(8)),
        trace=bool(os.environ.get("KERNEL_TRACE")),
        tmpdir=os.environ.get("KERNEL_TRACE_DIR") or None,
    )
    _CACHE["last_results"] = res

    out = np.zeros((B, T, D), dtype=np.float32)
    for b in range(B):
        acc = np.zeros((T, D), dtype=np.float32)
        for g in range(G):
            acc += res.results[b * G + g]["y"]
        out[b] = acc + bo[None, :]
    return out


# revision 12
# speedup vs baseline: 1.0532x; 1.0532x over previous
"""Grouped self-attention (GQA) Trainium2 kernel.

Problem: B=2, T=2048, D=2048, 16 Q heads / 4 KV heads, head_dim=128,
full RoPE (base 1e6), causal softmax, output projection.

Sharding: 8 cores = 2 batches x 4 KV groups. Core c handles batch c//4,
kv-group c%4 (4 Q heads + 1 KV head). q/k/v projections column-sharded,
o_proj row-sharded; per-core partial outputs are summed on host.

Per-core pipeline (all matmuls fp32r except P@V in bf16):
  phase 1: qT/kT/vT = W.T @ x.T (x pre-transposed on host), RoPE fused
           on PSUM->SBUF eviction, v transposed to [tk, d] blocks.
  phase 2: per 128-row q block i, per head h:
           S = qT_h.T @ kT (causal blocks only), mask diag, exp on ACT
           (scale=1/sqrt(d), accum_out=row sums), P^T via PE transpose,
           O = P^T.T @ V accumulated over tk blocks, normalize by
           1/rowsum on PSUM eviction, O^T via PE transpose, then
           Y_partial = O^T.T @ Wo_g streamed straight from PSUM to DRAM.
"""

import os
import sys

import numpy as np

for _p in ("/opt/trn_rl_repo",):
    if _p not in sys.path and os.path.isdir(_p):
        sys.path.insert(0, _p)

import concourse.bass as bass  # noqa: E402
import concourse.mybir as mybir  # noqa: E402
import concourse.tile as tile  # noqa: E402
from concourse import bacc  # noqa: E402
from concourse.bass_utils import run_bass_kernel_spmd  # noqa: E402
from concourse.masks import make_identity  # noqa: E402

B, T, D = 2, 2048, 2048
NH, NKV, HD = 16, 4, 128
G = NKV              # kv groups == cores per batch
AQ = (NH // NKV) * HD  # attention cols per core (4 heads x 128)
NQB = T // 128       # 16 q blocks
KC = D // 128        # 16 contraction chunks for projections
ROPE_BASE = 1000000.0
INV_SQRT_D = 1.0 / float(np.sqrt(HD))

F32 = mybir.dt.float32
F32R = mybir.dt.float32r
BF16 = mybir.dt.bfloat16

_CACHE = {}


def _build_nc():
    nc = bacc.Bacc(None, target_bir_lowering=False, debug=False)

    xT_d = nc.dram_tensor("xT", [D, T], F32R, kind="ExternalInput")
    wq_d = nc.dram_tensor("wq", [D, AQ], F32R, kind="ExternalInput")
    wk_d = nc.dram_tensor("wk", [D, HD], F32R, kind="ExternalInput")
    wv_d = nc.dram_tensor("wv", [D, HD], F32R, kind="ExternalInput")
    wo_d = nc.dram_tensor("wo", [AQ, D], F32R, kind="ExternalInput")
    cos_d = nc.dram_tensor("cosT", [HD, T], F32, kind="ExternalInput")
    sin_d = nc.dram_tensor("sinT", [HD, T], F32, kind="ExternalInput")
    mask_d = nc.dram_tensor("mask", [128, 128], F32, kind="ExternalInput")
    y_d = nc.dram_tensor("y", [T, D], F32, kind="ExternalOutput")

    mult = mybir.AluOpType.mult
    add = mybir.AluOpType.add
    Exp = mybir.ActivationFunctionType.Exp
    Copy = mybir.ActivationFunctionType.Copy

    with tile.TileContext(nc) as tc:
        with (
            tc.tile_pool(name="const", bufs=1) as cpool,
            tc.tile_pool(name="qkv", bufs=1) as qkv_pool,
        ):
            cos_sb = cpool.tile([HD, T], F32, tag="cos")
            sin_sb = cpool.tile([HD, T], F32, tag="sin")
            mask_sb = cpool.tile([128, 128], F32, tag="mask")
            id_bf = cpool.tile([128, 128], BF16, tag="idb")
            id_f32 = cpool.tile([128, 128], F32, tag="idf")
            nc.sync.dma_start(cos_sb[:], cos_d[:])
            nc.sync.dma_start(sin_sb[:], sin_d[:])
            nc.sync.dma_start(mask_sb[:], mask_d[:])
            make_identity(nc, id_bf[:])
            make_identity(nc, id_f32[:])

            qT = qkv_pool.tile([128, 4, T], F32R, tag="qT")    # [d, h, t]
            kT = qkv_pool.tile([128, T], F32R, tag="kT")       # [d, t]
            v_sb = qkv_pool.tile([128, T], BF16, tag="v")     # [tk%128, blk*128+d]

            # ---------------- phase 1: projections + rope ----------------
            with (
                tc.tile_pool(name="xt", bufs=1) as xt_pool,
                tc.tile_pool(name="wld", bufs=4) as w_pool,
                tc.tile_pool(name="p1ps", bufs=1, space="PSUM") as pps,
                tc.tile_pool(name="p1vt", bufs=2, space="PSUM") as pvt,
                tc.tile_pool(name="p1tmp", bufs=3) as tmp_pool,
            ):
                xt = xt_pool.tile([128, KC, T], F32R, tag="xt")
                for e in range(KC):
                    nc.sync.dma_start(xt[:, e, :], xT_d[e * 128:(e + 1) * 128, :])

                for s in range(6):
                    if s < 4:
                        src = wq_d[:, s * 128:(s + 1) * 128]
                    elif s == 4:
                        src = wk_d[:, :]
                    else:
                        src = wv_d[:, :]
                    pss = [pps.tile([128, 512], F32, tag=f"proj{t}",
                                    name=f"proj_{s}_{t}")
                           for t in range(4)]
                    for e in range(KC):
                        we = w_pool.tile([128, 128], F32R, tag="w")
                        nc.sync.dma_start(we[:], src[e * 128:(e + 1) * 128, :])
                        for tci in range(4):
                            nc.tensor.matmul(
                                pss[tci][:],
                                we[:],
                                xt[:, e, tci * 512:(tci + 1) * 512],
                                start=(e == 0),
                                stop=(e == KC - 1),
                            )
                    for tci in range(4):
                        tsl = slice(tci * 512, (tci + 1) * 512)
                        ps = pss[tci]
                        if s < 5:
                            dst = qT[:, s, tsl] if s < 4 else kT[:, tsl]
                            t1 = tmp_pool.tile([128, 512], F32, tag="ropetmp")
                            nc.vector.tensor_tensor(t1[:], ps[:], cos_sb[:, tsl], mult)
                            nc.vector.tensor_tensor(
                                dst[0:64, :], ps[64:128, :], sin_sb[0:64, tsl], mult)
                            nc.vector.tensor_tensor(
                                dst[64:128, :], ps[0:64, :], sin_sb[64:128, tsl], mult)
                            nc.vector.tensor_tensor(dst[:], dst[:], t1[:], add)
                        else:
                            # vT chunk [d, t512] -> bf16, then transpose to v blocks
                            vt = tmp_pool.tile([128, 512], BF16, tag="vtmp")
                            nc.scalar.copy(vt[:], ps[:])
                            pst = pvt.tile([128, 512], BF16, tag="vtr")
                            for j4 in range(4):
                                nc.tensor.transpose(
                                    pst[:, j4 * 128:(j4 + 1) * 128],
                                    vt[:, j4 * 128:(j4 + 1) * 128],
                                    id_bf[:],
                                )
                            nc.vector.tensor_copy(v_sb[:, tsl], pst[:])

            # ---------------- phase 2: attention + o-proj ----------------
            with (
                tc.tile_pool(name="wop", bufs=1) as wo_pool,
                tc.tile_pool(name="att", bufs=2) as att_pool,
                tc.tile_pool(name="small", bufs=6) as small_pool,
                tc.tile_pool(name="ps_s", bufs=2, space="PSUM") as ps_s_pool,
                tc.tile_pool(name="ps_t", bufs=2, space="PSUM") as ps_t_pool,
                tc.tile_pool(name="ps_o", bufs=2, space="PSUM") as ps_o_pool,
                tc.tile_pool(name="ps_y", bufs=2, space="PSUM") as ps_y_pool,
            ):
                wo_sb = wo_pool.tile([128, 4, D], F32R, tag="wo")
                for h in range(4):
                    nc.sync.dma_start(
                        wo_sb[:, h, :], wo_d[h * 128:(h + 1) * 128, :])

                cp = 0
                for i in range(NQB):
                    L = (i + 1) * 128
                    ncnk = (L + 511) // 512
                    O_sb = small_pool.tile([128, 4, 128], F32, tag="Osb")
                    for h in range(4):
                        P = att_pool.tile([128, T], BF16, tag="P")
                        sums = small_pool.tile([128, 4], F32, tag="sums")
                        for c in range(ncnk):
                            cl = min(512, L - c * 512)
                            ps_s = ps_s_pool.tile([128, 512], F32, tag="S")
                            nc.tensor.matmul(
                                ps_s[:, :cl],
                                qT[:, h, i * 128:(i + 1) * 128],
                                kT[:, c * 512:c * 512 + cl],
                                start=True,
                                stop=True,
                            )
                            if c == ncnk - 1:
                                nc.vector.tensor_tensor(
                                    ps_s[:, cl - 128:cl], ps_s[:, cl - 128:cl],
                                    mask_sb[:], add)
                            nc.scalar.activation(
                                P[:, c * 512:c * 512 + cl],
                                ps_s[:, :cl],
                                Exp,
                                scale=INV_SQRT_D,
                                accum_out=sums[:, c:c + 1],
                            )
                        rs = small_pool.tile([128, 1], F32, tag="rs")
                        if ncnk > 1:
                            tot = small_pool.tile([128, 1], F32, tag="tot")
                            nc.vector.tensor_reduce(
                                tot[:], sums[:, :ncnk],
                                axis=mybir.AxisListType.X, op=add)
                            nc.vector.reciprocal(rs[:], tot[:])
                        else:
                            nc.vector.reciprocal(rs[:], sums[:, 0:1])

                        PT = att_pool.tile([128, T], BF16, tag="PT")
                        for jb in range(0, i + 1, 4):
                            jn = min(4, i + 1 - jb)
                            ps_t = ps_t_pool.tile([128, 512], BF16, tag="PTb")
                            for j4 in range(jn):
                                nc.tensor.transpose(
                                    ps_t[:, j4 * 128:(j4 + 1) * 128],
                                    P[:, (jb + j4) * 128:(jb + j4 + 1) * 128],
                                    id_bf[:],
                                )
                            dst = PT[:, jb * 128:(jb + jn) * 128]
                            if cp % 2 == 0:
                                nc.scalar.copy(dst, ps_t[:, :jn * 128])
                            else:
                                nc.vector.tensor_copy(dst, ps_t[:, :jn * 128])
                            cp += 1

                        ps_o = ps_o_pool.tile([128, 128], F32, tag="O")
                        for j in range(i + 1):
                            nc.tensor.matmul(
                                ps_o[:],
                                PT[:, j * 128:(j + 1) * 128],
                                v_sb[:, j * 128:(j + 1) * 128],
                                start=(j == 0),
                                stop=(j == i),
                            )
                        nc.scalar.activation(
                            O_sb[:, h, :], ps_o[:], Copy, scale=rs[:])

                    ps_ot = ps_t_pool.tile([128, 512], F32, tag="PTp", bufs=1)
                    for h in range(4):
                        nc.tensor.transpose(
                            ps_ot[:, h * 128:(h + 1) * 128], O_sb[:, h, :],
                            id_f32[:])
                    OT_sb = small_pool.tile([128, 512], F32R, tag="OT")
                    nc.vector.tensor_copy(OT_sb[:], ps_ot[:])

                    for nci in range(4):
                        ps_y = ps_y_pool.tile([128, 512], F32, tag="Y", bufs=1)
                        for h in range(4):
                            nc.tensor.matmul(
                                ps_y[:],
                                OT_sb[:, h * 128:(h + 1) * 128],
                                wo_sb[:, h, nci * 512:(nci + 1) * 512],
                                start=(h == 0),
                                stop=(h == 3),
                            )
                        y_sb = att_pool.tile([128, 512], F32, tag="ysb")
                        if cp % 2 == 0:
                            nc.scalar.copy(y_sb[:], ps_y[:])
                        else:
                            nc.vector.tensor_copy(y_sb[:], ps_y[:])
                        cp += 1
                        nc.sync.dma_start(
                            y_d[i * 128:(i + 1) * 128, nci * 512:(nci + 1) * 512],
                            y_sb[:])

    nc.compile()
    return nc


def _rope_tables():
    # match reference float32 arithmetic exactly
    pos = np.arange(T, dtype=np.float32)
    inv_freq = (1.0 / (ROPE_BASE ** (np.arange(0, HD, 2, dtype=np.float32) / HD))).astype(np.float32)
    ang = pos[:, None] * inv_freq[None, :]            # [T, 64]
    cos = np.cos(ang).astype(np.float32)
    sin = np.sin(ang).astype(np.float32)
    cosT = np.ascontiguousarray(np.concatenate([cos, cos], 1).T)   # [128, T]
    sinT = np.ascontiguousarray(np.concatenate([-sin, sin], 1).T)  # rotate_half sign
    return cosT, sinT


def kernel(x, Wq, bq, Wk, bk, Wv, bv, Wo, bo, **_ignored):
    x = np.asarray(x, dtype=np.float32)
    Wq = np.asarray(Wq, dtype=np.float32)
    Wk = np.asarray(Wk, dtype=np.float32)
    Wv = np.asarray(Wv, dtype=np.float32)
    Wo = np.asarray(Wo, dtype=np.float32)
    bo = np.asarray(bo, dtype=np.float32)

    if "nc" not in _CACHE:
        _CACHE["nc"] = _build_nc()
    nc = _CACHE["nc"]

    cosT, sinT = _rope_tables()
    tri = np.tril(np.ones((128, 128), dtype=bool))
    mask = np.where(tri, 0.0, -1e9).astype(np.float32)

    in_maps = []
    for c in range(8):
        b, g = c // G, c % G
        in_maps.append({
            "xT": np.ascontiguousarray(x[b].T),
            "wq": np.ascontiguousarray(Wq[:, g * AQ:(g + 1) * AQ]),
            "wk": np.ascontiguousarray(Wk[:, g * HD:(g + 1) * HD]),
            "wv": np.ascontiguousarray(Wv[:, g * HD:(g + 1) * HD]),
            "wo": np.ascontiguousarray(Wo[g * AQ:(g + 1) * AQ, :]),
            "cosT": cosT,
            "sinT": sinT,
            "mask": mask,
        })

    res = run_bass_kernel_spmd(
        nc, in_maps, list(range(8)),
        trace=bool(os.environ.get("KERNEL_TRACE")),
        tmpdir=os.environ.get("KERNEL_TRACE_DIR") or None,
    )
    _CACHE["last_results"] = res

    out = np.zeros((B, T, D), dtype=np.float32)
    for b in range(B):
        acc = np.zeros((T, D), dtype=np.float32)
        for g in range(G):
            acc += res.results[b * G + g]["y"]
        out[b] = acc + bo[None, :]
    return out
